# revision 12
# baseline (speedup 1.0000x reference)
"""Trainium2 Bass kernel for nn_BoxDetectionLoss (8-core data parallel).

Math: reference loss = sum_{a,r,c}[ has_match ? coord+conf_loss : conf^2 ] / denom.
A pixel (r,c) can only match a target box t if r==tb[t,0] and c==tb[t,1]
(T=16 boxes per image), so the dense term is just sum sigmoid(conf_ch)^2 over
channels {2,5,8}; the match term is a correction at <=16 pixels x 3 anchors,
computed from 144 gathered elements per image.

Each of the 8 cores handles one batch image. v4 layout (DMA-roofline oriented):
  - The 3 conf channels stream in as 12 column chunks (512 cols = one DRAM row
    per partition, 2KB descriptors), ALL on the sync HWDGE queue so the
    Activation stream carries zero DMA generation work (HWDGE gen on the ACT
    sequencer was observed to stall the first sigmoid by ~8us).
  - ACT does sigmoid f32->bf16 in groups (512-col at the pipeline ends for
    fast fill/drain, 1024-col in the middle to amortize the ~230ns/op
    overhead); DVE does one fused tensor_tensor_reduce (square + row-sum into
    its own ACC column) per 512-col chunk. Compute pipelines under the DMA
    wall instead of serializing after it.
  - Correction in a [48,1] partition-parallel layout (partition = box*3+anchor)
    so every elementwise step is a tensor_scalar with per-partition AP scalars
    — the one form the Pool (GpSimd) engine supports — and the whole chain
    runs on the otherwise-idle Pool engine, hidden under the DMA window. The
    last op writes the 48 contributions straight into ACC rows (host sums).
  - Only one ACT table load (the compiler's redundant set-0 load at entry is
    retargeted to the sigmoid set post-compile, the duplicate dropped).
  - Output: ACC [128,13] DMA'd straight out on the scalar queue; host sums.
"""

import os
import numpy as np

CORR_ENGINE = os.environ.get("KV_CORR", "vector")

B, C, H, W = 8, 9, 512, 512
T = 16
N_CORES = 8
CONF_CH = (2, 5, 8)
DENOM = float(B * H * W * 3)
MAGIC = 12582912.0  # 1.5 * 2^23: x+MAGIC-MAGIC rounds to nearest-even int
NCHUNK = 4          # DMA chunks per channel (512 cols each)
CHUNK = 2048 // NCHUNK
TA = 3 * T          # correction partitions: (box, anchor)

# sigmoid op granularity: (channel, start, len) — small at the ends for fast
# pipeline fill/drain, large in the middle
SIG_GROUPS = [
    (0, 0, 512), (0, 512, 512), (0, 1024, 1024),
    (1, 0, 1024), (1, 1024, 1024),
    (2, 0, 1024), (2, 1024, 512), (2, 1536, 512),
]

_PROG = None


def _build_correction_load(nc, sp, bass, mybir, bund, pol):
    f32 = mybir.dt.float32
    i32 = mybir.dt.int32

    BUND = sp.tile([TA, 16], f32)
    nc.gpsimd.dma_start(BUND[:], bund[:])
    OFF = sp.tile([TA, 3], i32)
    nc.vector.tensor_copy(OFF[:], BUND[:, 7:10].bitcast(i32))
    G = sp.tile([TA, 3], f32)

    def emit_gather():
        nc.gpsimd.indirect_dma_start(
            out=G[:], out_offset=None,
            in_=pol.rearrange("c h (w a) -> (c h w) a", a=1),
            in_offset=bass.IndirectOffsetOnAxis(ap=OFF[:], axis=0),
        )
    return dict(BUND=BUND, G=G, emit_gather=emit_gather)


def _build_correction_compute(nc, sp, ACC, mybir, ctx):
    """[48,1] chain on the Pool engine: partition p = box*3 + anchor; every
    binary op is tensor_scalar with a per-partition AP scalar (the only
    elementwise form Pool supports). Result lands in ACC[0:48, NDCOL]."""
    f32 = mybir.dt.float32
    ALU = mybir.AluOpType
    BUND, GS = ctx["BUND"], ctx["GS"]
    TB0, TB1 = BUND[:, 0:1], BUND[:, 1:2]
    TB2, TB3 = BUND[:, 2:3], BUND[:, 3:4]
    TP, KEEP = BUND[:, 4:5], BUND[:, 5:6]
    SR, SC, SF = GS[:, 0:1], GS[:, 1:2], GS[:, 2:3]

    ts = (nc.gpsimd if CORR_ENGINE == "pool" else nc.vector).tensor_scalar
    predr = sp.tile([TA, 1], f32)
    ts(out=predr[:], in0=SR, scalar1=9.0, scalar2=TB0, op0=ALU.mult, op1=ALU.add)
    ts(out=predr[:], in0=predr[:], scalar1=511.0, scalar2=None, op0=ALU.min)
    rr = sp.tile([TA, 1], f32)
    ts(out=rr[:], in0=predr[:], scalar1=MAGIC, scalar2=MAGIC,
       op0=ALU.add, op1=ALU.subtract)
    predc = sp.tile([TA, 1], f32)
    ts(out=predc[:], in0=SC, scalar1=16.0, scalar2=TB1, op0=ALU.mult, op1=ALU.add)
    ts(out=predc[:], in0=predc[:], scalar1=511.0, scalar2=None, op0=ALU.min)
    rc = sp.tile([TA, 1], f32)
    ts(out=rc[:], in0=predc[:], scalar1=MAGIC, scalar2=MAGIC,
       op0=ALU.add, op1=ALU.subtract)

    m = sp.tile([TA, 1], f32)
    ts(out=m[:], in0=rr[:], scalar1=TB2, scalar2=KEEP,
       op0=ALU.is_equal, op1=ALU.mult)
    m2 = sp.tile([TA, 1], f32)
    ts(out=m2[:], in0=rc[:], scalar1=TB3, scalar2=None, op0=ALU.is_equal)
    ts(out=m[:], in0=m[:], scalar1=m2[:], scalar2=None, op0=ALU.mult)

    # |x| as max(x, -x)
    d1 = sp.tile([TA, 1], f32)
    ts(out=d1[:], in0=predr[:], scalar1=TB2, scalar2=None, op0=ALU.subtract)
    d1n = sp.tile([TA, 1], f32)
    ts(out=d1n[:], in0=d1[:], scalar1=-1.0, scalar2=None, op0=ALU.mult)
    ts(out=d1[:], in0=d1[:], scalar1=d1n[:], scalar2=None, op0=ALU.max)
    d2 = sp.tile([TA, 1], f32)
    ts(out=d2[:], in0=predc[:], scalar1=TB3, scalar2=None, op0=ALU.subtract)
    d2n = sp.tile([TA, 1], f32)
    ts(out=d2n[:], in0=d2[:], scalar1=-1.0, scalar2=None, op0=ALU.mult)
    ts(out=d2[:], in0=d2[:], scalar1=d2n[:], scalar2=None, op0=ALU.max)
    ts(out=d1[:], in0=d1[:], scalar1=d2[:], scalar2=None, op0=ALU.add)

    cf = sp.tile([TA, 1], f32)
    ts(out=cf[:], in0=SF, scalar1=-2.0, scalar2=TP, op0=ALU.mult, op1=ALU.add)
    ts(out=cf[:], in0=cf[:], scalar1=TP, scalar2=None, op0=ALU.mult)
    ts(out=d1[:], in0=d1[:], scalar1=cf[:], scalar2=None, op0=ALU.add)
    ts(out=ACC[0:TA, 1:2], in0=m[:], scalar1=d1[:], scalar2=None,
       op0=ALU.mult)


def _build_program():
    import concourse.bass as bass
    import concourse.tile as tile
    from concourse import bacc, mybir

    f32 = mybir.dt.float32
    bf16 = mybir.dt.bfloat16
    ALU = mybir.AluOpType
    ACT_F = mybir.ActivationFunctionType

    nc = bacc.Bacc(
        "TRN2", target_bir_lowering=False, debug=False, num_devices=N_CORES
    )
    pol = nc.dram_tensor("pol", [C, H, W], f32, kind="ExternalInput").ap()
    bund = nc.dram_tensor("bund", [TA, 16], f32, kind="ExternalInput").ap()
    out = nc.dram_tensor("out", [TA, 2], f32, kind="ExternalOutput").ap()

    with tile.TileContext(nc) as tc:
        with (
            tc.tile_pool(name="p", bufs=1) as sp,
            tc.tile_pool(name="ps", bufs=1, space="PSUM") as psum,
        ):
            views = [
                pol[ch].rearrange("(p a) w -> p (a w)", p=128) for ch in CONF_CH
            ]
            tin = [sp.tile([128, 2048], f32, name=f"tin{i}", tag=f"t{i}")
                   for i in range(3)]
            sig = [sp.tile([128, 2048], bf16, name=f"sig{i}", tag=f"s{i}")
                   for i in range(3)]
            sq = [sp.tile([128, 2048], bf16, name=f"sq{i}", tag=f"q{i}")
                  for i in range(3)]

            # dense chunks split 6/6 across the sync HWDGE queue and the
            # gpsimd SWDGE ring (one queue alone paces at ~1us/chunk, two
            # queues keep the 16 DMA engines ~full). The SWDGE ring order is
            # bund -> 2 chunks -> gather -> 4 chunks so the tiny gather isn't
            # stuck behind all the dense traffic.
            corr_ctx = _build_correction_load(nc, sp, bass, mybir, bund, pol)

            def chunk_dma(eng, ci, k):
                cs = slice(k * CHUNK, (k + 1) * CHUNK)
                eng.dma_start(tin[ci][:, cs], views[ci][:, cs])

            ACC = sp.tile([TA, 2], f32)
            nc.gpsimd.memset(ACC[:], 0.0)
            ONES = sp.tile([128, 1], bf16)
            nc.gpsimd.memset(ONES[:], 1.0)

            order = [(ci, k) for ci in range(3) for k in range(NCHUNK)]
            for ci, k in order:
                if k % 2 == 0:
                    chunk_dma(nc.sync, ci, k)
            swdge = [(ci, k) for ci, k in order if k % 2 == 1]
            for ci, k in swdge[:2]:
                chunk_dma(nc.gpsimd, ci, k)
            corr_ctx["emit_gather"]()
            for ci, k in swdge[2:]:
                chunk_dma(nc.gpsimd, ci, k)
            PS = psum.tile([1, 2048 // NCHUNK], f32, space="PSUM")
            NMM = 3 * NCHUNK
            mm_idx = [0]

            def do_sq(ci, k):
                # DVE square (bf16, ~400ns), then the idle PE accumulates the
                # row-sums: ones[128,1].T @ sq[128,512] -> PSUM [1,512],
                # accumulated across all 12 chunks in one bank
                cs = slice(k * CHUNK, (k + 1) * CHUNK)
                nc.vector.tensor_tensor(
                    out=sq[ci][:, cs], in0=sig[ci][:, cs], in1=sig[ci][:, cs],
                    op=ALU.mult,
                )
                i = mm_idx[0]
                mm_idx[0] += 1
                nc.tensor.matmul(out=PS[:], lhsT=ONES[:], rhs=sq[ci][:, cs],
                                 start=(i == 0), stop=(i == NMM - 1))

            GS = sp.tile([TA, 3], f32)
            corr_ctx["GS"] = GS
            for gi, (ci, start, ln) in enumerate(SIG_GROUPS):
                cs = slice(start, start + ln)
                nc.scalar.activation(sig[ci][:, cs], tin[ci][:, cs], ACT_F.Sigmoid)
                for k in range(start // CHUNK, (start + ln) // CHUNK):
                    do_sq(ci, k)
                if gi == 3:
                    # tiny correction sigmoid: gather lands mid-window
                    nc.scalar.activation(GS[:], corr_ctx["G"][:], ACT_F.Sigmoid)
                if gi == 4:
                    # DVE correction chain, slotted after the 6th chunk's
                    # square so it neither blocks the early squares nor
                    # lands on the tail
                    _build_correction_compute(nc, sp, ACC, mybir, corr_ctx)

            # collapse the accumulated [1,512] PSUM row into ACC[0,0]
            nc.vector.tensor_reduce(
                out=ACC[0:1, 0:1], in_=PS[:], axis=mybir.AxisListType.X,
                op=ALU.add,
            )
            nc.scalar.dma_start(out[:], ACC[:])

    nc.compile()

    # drop the compiler's redundant entry-time set-0 ACT table load: retarget
    # it to the sigmoid set (2) and remove the duplicate set-2 load (1.5us
    # of ACT stream head each; both carry no sync_info)
    for func in nc.m.functions:
        for block in func.blocks:
            loads = [i for i in block.instructions
                     if isinstance(i, mybir.InstLoadActFuncSet)]
            if len(loads) == 2 and all(i.sync_info is None for i in loads):
                loads[0].act_func_set_id = 2
                block.instructions.remove(loads[1])
    return nc


def get_program():
    global _PROG
    if _PROG is None:
        _PROG = _build_program()
    return _PROG


def _make_bundle(tb, tpv):
    """[48,16] f32, partition p = box*3 + anchor: tb as f32 (cols 0-3),
    tp (4), dup-keep (5), pad (6), the 3 gather offsets (3a+k)*H*W + r*W + c
    for k=0,1,2 as int32 bits (7-9)."""
    tb = tb.astype(np.int64)
    bundle = np.zeros((TA, 16), dtype=np.float32)
    rep = np.repeat(np.arange(T), 3)          # box index per partition
    anc = np.tile(np.arange(3), T)            # anchor index per partition
    bundle[:, 0:4] = tb[rep].astype(np.float32)
    bundle[:, 4] = tpv[rep]
    dup = np.zeros(T, dtype=bool)
    for t in range(1, T):
        dup[t] = (tb[:t] == tb[t]).all(axis=1).any()
    bundle[:, 5] = (~dup).astype(np.float32)[rep]
    p1 = tb[:, 0] * W + tb[:, 1]
    goff = ((3 * anc[:, None] + np.arange(3)[None, :]) * (H * W)
            + p1[rep][:, None])
    bundle[:, 7:10] = goff.astype(np.int32).view(np.float32)
    return bundle


def make_in_maps(policy_output, target_boxes, target_probs):
    policy_output = np.ascontiguousarray(np.asarray(policy_output, dtype=np.float32))
    target_boxes = np.ascontiguousarray(np.asarray(target_boxes, dtype=np.int32))
    target_probs = np.ascontiguousarray(np.asarray(target_probs, dtype=np.float32))
    assert policy_output.shape == (B, C, H, W)
    in_maps = []
    for i in range(N_CORES):
        in_maps.append(
            {
                "pol": policy_output[i],
                "bund": _make_bundle(target_boxes[i], target_probs[i]),
            }
        )
    return in_maps


def kernel(policy_output, target_boxes, target_probs):
    from concourse.bass_utils import run_bass_kernel_spmd

    nc = get_program()
    in_maps = make_in_maps(policy_output, target_boxes, target_probs)
    res = run_bass_kernel_spmd(nc, in_maps, list(range(N_CORES)))
    total = 0.0
    for i in range(N_CORES):
        total += float(res.results[i]["out"].sum(dtype=np.float64))
    return np.float32(total / DENOM)


# revision 13
# speedup vs baseline: 1.0415x; 1.0415x over previous
"""Trainium2 Bass kernel for nn_BoxDetectionLoss (8-core data parallel).

Math: reference loss = sum_{a,r,c}[ has_match ? coord+conf_loss : conf^2 ] / denom.
A pixel (r,c) can only match a target box t if r==tb[t,0] and c==tb[t,1]
(T=16 boxes per image), so the dense term is just sum sigmoid(conf_ch)^2 over
channels {2,5,8}; the match term is a correction at <=16 pixels x 3 anchors,
computed from 144 gathered elements per image.

Each of the 8 cores handles one batch image. v4 layout (DMA-roofline oriented):
  - The 3 conf channels stream in as 12 column chunks (512 cols = one DRAM row
    per partition, 2KB descriptors), ALL on the sync HWDGE queue so the
    Activation stream carries zero DMA generation work (HWDGE gen on the ACT
    sequencer was observed to stall the first sigmoid by ~8us).
  - ACT does sigmoid f32->bf16 in groups (512-col at the pipeline ends for
    fast fill/drain, 1024-col in the middle to amortize the ~230ns/op
    overhead); DVE does one fused tensor_tensor_reduce (square + row-sum into
    its own ACC column) per 512-col chunk. Compute pipelines under the DMA
    wall instead of serializing after it.
  - Correction in a [48,1] partition-parallel layout (partition = box*3+anchor)
    so every elementwise step is a tensor_scalar with per-partition AP scalars
    — the one form the Pool (GpSimd) engine supports — and the whole chain
    runs on the otherwise-idle Pool engine, hidden under the DMA window. The
    last op writes the 48 contributions straight into ACC rows (host sums).
  - Only one ACT table load (the compiler's redundant set-0 load at entry is
    retargeted to the sigmoid set post-compile, the duplicate dropped).
  - Output: ACC [128,13] DMA'd straight out on the scalar queue; host sums.
"""

import os
import numpy as np

CORR_ENGINE = os.environ.get("KV_CORR", "vector")

B, C, H, W = 8, 9, 512, 512
T = 16
N_CORES = 8
CONF_CH = (2, 5, 8)
DENOM = float(B * H * W * 3)
MAGIC = 12582912.0  # 1.5 * 2^23: x+MAGIC-MAGIC rounds to nearest-even int
NCHUNK = 4          # DMA chunks per channel (512 cols each)
CHUNK = 2048 // NCHUNK
TA = 3 * T          # correction partitions: (box, anchor)

# sigmoid op granularity: (channel, start, len) — small at the ends for fast
# pipeline fill/drain, large in the middle
SIG_GROUPS = [
    (0, 0, 512), (0, 512, 512), (0, 1024, 1024),
    (1, 0, 1024), (1, 1024, 1024),
    (2, 0, 1024), (2, 1024, 512), (2, 1536, 512),
]

_PROG = None


def _build_correction_load(nc, sp, bass, mybir, bund, pol):
    f32 = mybir.dt.float32
    i32 = mybir.dt.int32

    BUND = sp.tile([TA, 16], f32)
    nc.gpsimd.dma_start(BUND[:], bund[:])
    OFF = sp.tile([TA, 3], i32)
    nc.vector.tensor_copy(OFF[:], BUND[:, 7:10].bitcast(i32))
    G = sp.tile([TA, 3], f32)

    def emit_gather():
        nc.gpsimd.indirect_dma_start(
            out=G[:], out_offset=None,
            in_=pol.rearrange("c h (w a) -> (c h w) a", a=1),
            in_offset=bass.IndirectOffsetOnAxis(ap=OFF[:], axis=0),
        )
    return dict(BUND=BUND, G=G, emit_gather=emit_gather)


def _build_correction_compute(nc, sp, ACC, mybir, ctx):
    """[48,1] chain on the Pool engine: partition p = box*3 + anchor; every
    binary op is tensor_scalar with a per-partition AP scalar (the only
    elementwise form Pool supports). Result lands in ACC[0:48, NDCOL]."""
    f32 = mybir.dt.float32
    ALU = mybir.AluOpType
    BUND, GS = ctx["BUND"], ctx["GS"]
    TB0, TB1 = BUND[:, 0:1], BUND[:, 1:2]
    TB2, TB3 = BUND[:, 2:3], BUND[:, 3:4]
    TP, KEEP = BUND[:, 4:5], BUND[:, 5:6]
    SR, SC, SF = GS[:, 0:1], GS[:, 1:2], GS[:, 2:3]

    ts = (nc.gpsimd if CORR_ENGINE == "pool" else nc.vector).tensor_scalar
    predr = sp.tile([TA, 1], f32)
    ts(out=predr[:], in0=SR, scalar1=9.0, scalar2=TB0, op0=ALU.mult, op1=ALU.add)
    ts(out=predr[:], in0=predr[:], scalar1=511.0, scalar2=None, op0=ALU.min)
    rr = sp.tile([TA, 1], f32)
    ts(out=rr[:], in0=predr[:], scalar1=MAGIC, scalar2=MAGIC,
       op0=ALU.add, op1=ALU.subtract)
    predc = sp.tile([TA, 1], f32)
    ts(out=predc[:], in0=SC, scalar1=16.0, scalar2=TB1, op0=ALU.mult, op1=ALU.add)
    ts(out=predc[:], in0=predc[:], scalar1=511.0, scalar2=None, op0=ALU.min)
    rc = sp.tile([TA, 1], f32)
    ts(out=rc[:], in0=predc[:], scalar1=MAGIC, scalar2=MAGIC,
       op0=ALU.add, op1=ALU.subtract)

    m = sp.tile([TA, 1], f32)
    ts(out=m[:], in0=rr[:], scalar1=TB2, scalar2=KEEP,
       op0=ALU.is_equal, op1=ALU.mult)
    m2 = sp.tile([TA, 1], f32)
    ts(out=m2[:], in0=rc[:], scalar1=TB3, scalar2=None, op0=ALU.is_equal)
    ts(out=m[:], in0=m[:], scalar1=m2[:], scalar2=None, op0=ALU.mult)

    # |x| as max(x, -x)
    d1 = sp.tile([TA, 1], f32)
    ts(out=d1[:], in0=predr[:], scalar1=TB2, scalar2=None, op0=ALU.subtract)
    d1n = sp.tile([TA, 1], f32)
    ts(out=d1n[:], in0=d1[:], scalar1=-1.0, scalar2=None, op0=ALU.mult)
    ts(out=d1[:], in0=d1[:], scalar1=d1n[:], scalar2=None, op0=ALU.max)
    d2 = sp.tile([TA, 1], f32)
    ts(out=d2[:], in0=predc[:], scalar1=TB3, scalar2=None, op0=ALU.subtract)
    d2n = sp.tile([TA, 1], f32)
    ts(out=d2n[:], in0=d2[:], scalar1=-1.0, scalar2=None, op0=ALU.mult)
    ts(out=d2[:], in0=d2[:], scalar1=d2n[:], scalar2=None, op0=ALU.max)
    ts(out=d1[:], in0=d1[:], scalar1=d2[:], scalar2=None, op0=ALU.add)

    cf = sp.tile([TA, 1], f32)
    ts(out=cf[:], in0=SF, scalar1=-2.0, scalar2=TP, op0=ALU.mult, op1=ALU.add)
    ts(out=cf[:], in0=cf[:], scalar1=TP, scalar2=None, op0=ALU.mult)
    ts(out=d1[:], in0=d1[:], scalar1=cf[:], scalar2=None, op0=ALU.add)
    ts(out=ACC[0:TA, 1:2], in0=m[:], scalar1=d1[:], scalar2=None,
       op0=ALU.mult)


def _build_program():
    import concourse.bass as bass
    import concourse.tile as tile
    from concourse import bacc, mybir

    f32 = mybir.dt.float32
    bf16 = mybir.dt.bfloat16
    ALU = mybir.AluOpType
    ACT_F = mybir.ActivationFunctionType

    nc = bacc.Bacc(
        "TRN2", target_bir_lowering=False, debug=False, num_devices=N_CORES
    )
    pol = nc.dram_tensor("pol", [C, H, W], f32, kind="ExternalInput").ap()
    bund = nc.dram_tensor("bund", [TA, 16], f32, kind="ExternalInput").ap()
    out = nc.dram_tensor("out", [TA, 2], f32, kind="ExternalOutput").ap()

    with tile.TileContext(nc) as tc:
        with (
            tc.tile_pool(name="p", bufs=1) as sp,
            tc.tile_pool(name="ps", bufs=1, space="PSUM") as psum,
        ):
            views = [
                pol[ch].rearrange("(p a) w -> p (a w)", p=128) for ch in CONF_CH
            ]
            tin = [sp.tile([128, 2048], f32, name=f"tin{i}", tag=f"t{i}")
                   for i in range(3)]
            sig = [sp.tile([128, 2048], bf16, name=f"sig{i}", tag=f"s{i}")
                   for i in range(3)]
            sq = [sp.tile([128, 2048], bf16, name=f"sq{i}", tag=f"q{i}")
                  for i in range(3)]

            # dense chunks split 6/6 across the sync HWDGE queue and the
            # gpsimd SWDGE ring (one queue alone paces at ~1us/chunk, two
            # queues keep the 16 DMA engines ~full). The SWDGE ring order is
            # bund -> 2 chunks -> gather -> 4 chunks so the tiny gather isn't
            # stuck behind all the dense traffic.
            corr_ctx = _build_correction_load(nc, sp, bass, mybir, bund, pol)

            def chunk_dma(eng, ci, k):
                cs = slice(k * CHUNK, (k + 1) * CHUNK)
                eng.dma_start(tin[ci][:, cs], views[ci][:, cs])

            ACC = sp.tile([TA, 2], f32)
            nc.gpsimd.memset(ACC[:], 0.0)
            ONES = sp.tile([128, 1], bf16)
            nc.gpsimd.memset(ONES[:], 1.0)

            order = [(ci, k) for ci in range(3) for k in range(NCHUNK)]
            for ci, k in order:
                if k % 2 == 0:
                    chunk_dma(nc.sync, ci, k)
            # gather first in the SWDGE ring: its wait on the offset copy
            # (~3us) costs nothing — the dense swdge gens follow right after,
            # and the gather's 144 descriptors beat the dense flood
            corr_ctx["emit_gather"]()
            for ci, k in order:
                if k % 2 == 1:
                    chunk_dma(nc.gpsimd, ci, k)
            PS = psum.tile([1, 2048 // NCHUNK], f32, space="PSUM")
            NMM = 3 * NCHUNK
            mm_idx = [0]

            def do_sq(ci, k):
                # DVE square (bf16, ~400ns), then the idle PE accumulates the
                # row-sums: ones[128,1].T @ sq[128,512] -> PSUM [1,512],
                # accumulated across all 12 chunks in one bank
                cs = slice(k * CHUNK, (k + 1) * CHUNK)
                nc.vector.tensor_tensor(
                    out=sq[ci][:, cs], in0=sig[ci][:, cs], in1=sig[ci][:, cs],
                    op=ALU.mult,
                )
                i = mm_idx[0]
                mm_idx[0] += 1
                nc.tensor.matmul(out=PS[:], lhsT=ONES[:], rhs=sq[ci][:, cs],
                                 start=(i == 0), stop=(i == NMM - 1))

            GS = sp.tile([TA, 3], f32)
            corr_ctx["GS"] = GS
            for gi, (ci, start, ln) in enumerate(SIG_GROUPS):
                cs = slice(start, start + ln)
                nc.scalar.activation(sig[ci][:, cs], tin[ci][:, cs], ACT_F.Sigmoid)
                for k in range(start // CHUNK, (start + ln) // CHUNK):
                    do_sq(ci, k)
                if gi == 3:
                    # tiny correction sigmoid: gather lands mid-window
                    nc.scalar.activation(GS[:], corr_ctx["G"][:], ACT_F.Sigmoid)
                if gi == 4:
                    # DVE correction chain, slotted after the 6th chunk's
                    # square so it neither blocks the early squares nor
                    # lands on the tail
                    _build_correction_compute(nc, sp, ACC, mybir, corr_ctx)

            # collapse the accumulated [1,512] PSUM row into ACC[0,0]
            nc.vector.tensor_reduce(
                out=ACC[0:1, 0:1], in_=PS[:], axis=mybir.AxisListType.X,
                op=ALU.add,
            )
            nc.scalar.dma_start(out[:], ACC[:])

    nc.compile()

    # drop the compiler's redundant entry-time set-0 ACT table load: retarget
    # it to the sigmoid set (2) and remove the duplicate set-2 load (1.5us
    # of ACT stream head each; both carry no sync_info)
    for func in nc.m.functions:
        for block in func.blocks:
            loads = [i for i in block.instructions
                     if isinstance(i, mybir.InstLoadActFuncSet)]
            if len(loads) == 2 and all(i.sync_info is None for i in loads):
                loads[0].act_func_set_id = 2
                block.instructions.remove(loads[1])
    return nc


def get_program():
    global _PROG
    if _PROG is None:
        _PROG = _build_program()
    return _PROG


def _make_bundle(tb, tpv):
    """[48,16] f32, partition p = box*3 + anchor: tb as f32 (cols 0-3),
    tp (4), dup-keep (5), pad (6), the 3 gather offsets (3a+k)*H*W + r*W + c
    for k=0,1,2 as int32 bits (7-9)."""
    tb = tb.astype(np.int64)
    bundle = np.zeros((TA, 16), dtype=np.float32)
    rep = np.repeat(np.arange(T), 3)          # box index per partition
    anc = np.tile(np.arange(3), T)            # anchor index per partition
    bundle[:, 0:4] = tb[rep].astype(np.float32)
    bundle[:, 4] = tpv[rep]
    dup = np.zeros(T, dtype=bool)
    for t in range(1, T):
        dup[t] = (tb[:t] == tb[t]).all(axis=1).any()
    bundle[:, 5] = (~dup).astype(np.float32)[rep]
    p1 = tb[:, 0] * W + tb[:, 1]
    goff = ((3 * anc[:, None] + np.arange(3)[None, :]) * (H * W)
            + p1[rep][:, None])
    bundle[:, 7:10] = goff.astype(np.int32).view(np.float32)
    return bundle


def make_in_maps(policy_output, target_boxes, target_probs):
    policy_output = np.ascontiguousarray(np.asarray(policy_output, dtype=np.float32))
    target_boxes = np.ascontiguousarray(np.asarray(target_boxes, dtype=np.int32))
    target_probs = np.ascontiguousarray(np.asarray(target_probs, dtype=np.float32))
    assert policy_output.shape == (B, C, H, W)
    in_maps = []
    for i in range(N_CORES):
        in_maps.append(
            {
                "pol": policy_output[i],
                "bund": _make_bundle(target_boxes[i], target_probs[i]),
            }
        )
    return in_maps


def kernel(policy_output, target_boxes, target_probs):
    from concourse.bass_utils import run_bass_kernel_spmd

    nc = get_program()
    in_maps = make_in_maps(policy_output, target_boxes, target_probs)
    res = run_bass_kernel_spmd(nc, in_maps, list(range(N_CORES)))
    total = 0.0
    for i in range(N_CORES):
        total += float(res.results[i]["out"].sum(dtype=np.float64))
    return np.float32(total / DENOM)
